# revision 34
# baseline (speedup 1.0000x reference)
"""Trainium2 Bass kernel for nn_CDRsAttention (sparse multi-head attention
with padding mask + CDR key mask on the first 2 heads).

Sharding: 8 cores = 4 samples (B) x 2 head groups. Core (b, g) computes
heads [g, g+2, g+4, g+6] of sample b (exactly one CDR head each), producing
a partial output ctx_heads @ Wo_rows; the host sums the two partials + bo.

Host-side prep (pure numpy, cheap):
  - per-sample key gather: only keys with mask==1 participate, CDR-valid
    keys first, then regular keys, zero-padded to NK = 128*ceil(max valid).
    No inter-region padding: the CDR head attends tiles [0, NKT_CDR) and
    masks intruding regular keys via a per-partition additive bias (-30)
    fed to the exp activation for the boundary tiles.
  - xkT carries one extra row (valid-key indicator) that flows through
    augmented Wv selector columns so v column h*65+64 is the indicator row,
    making ctx^T row 64 the softmax denominator (padded keys drop out).
  - q/k biases are zero, so their projections contract over exactly 512
    rows (no augmented row).

Device per core (bf16 matmuls):
  qT/kT/v projections -> per head: S^T = kT_tile^T @ qT (keys on psum
  partitions), P = exp(S^T/8) on ScalarE straight out of PSUM (pairs of
  ktiles per activate), ctx^T accumulated as v_aug^T @ P, then a
  fast-reciprocal of the denominator row, a PE broadcast matmul (f32r),
  a DVE normalization multiply, and the output projection
  out = ctx_norm^T.T @ Wo_rows streamed straight to DRAM per 128-query
  slab. q/out projections are interleaved as PE filler between attention
  groups to keep the PE array dense and HAM-warm.
"""
import math
from contextlib import ExitStack

import ml_dtypes
import numpy as np

import concourse.bass as bass
import concourse.mybir as mybir
import concourse.tile as tile
from concourse import bacc
from concourse.bass_utils import run_bass_kernel_spmd

B, T, C, H, D = 4, 2048, 512, 8, 64
F32 = mybir.dt.float32
F32R = mybir.dt.float32r
BF16 = mybir.dt.bfloat16
EXP_SCALE = 1.0 / 8.0  # 1/sqrt(D)
MASK_BIAS = -30.0

_PROGRAM_CACHE: dict = {}
LAST_RESULTS = None  # BassKernelResults of the most recent kernel() call


def _chunks(total, step):
    return [(i, min(step, total - i)) for i in range(0, total, step)]


def _build_program(NKT, NKT_CDR, BIAS0, NBT):
    NK = NKT * 128
    nc = bacc.Bacc("TRN2", target_bir_lowering=False, debug=False, num_devices=8)
    xT_d = nc.dram_tensor("xT", [C, T], BF16, kind="ExternalInput").ap()
    xkT_d = nc.dram_tensor("xkT", [C + 1, NK], BF16, kind="ExternalInput").ap()
    wq_d = nc.dram_tensor("Wq", [C, 256], BF16, kind="ExternalInput").ap()
    wk_d = nc.dram_tensor("Wk", [C, 256], BF16, kind="ExternalInput").ap()
    wv_d = nc.dram_tensor("Wv", [C + 1, 260], BF16, kind="ExternalInput").ap()
    wo_d = nc.dram_tensor("Wo", [256, 512], BF16, kind="ExternalInput").ap()
    cdrb_d = nc.dram_tensor("cdrb", [128, max(NBT, 1)], F32,
                            kind="ExternalInput").ap()
    out_d = nc.dram_tensor("out", [T, 512], BF16, kind="ExternalOutput").ap()

    with tile.TileContext(nc) as tc:
        with ExitStack() as ctx:
            _body(ctx, tc, xT_d, xkT_d, wq_d, wk_d, wv_d, wo_d, cdrb_d, out_d,
                  NK, NKT, NKT_CDR, BIAS0, NBT)
    nc.compile()
    return nc


def _body(ctx, tc, xT_d, xkT_d, wq_d, wk_d, wv_d, wo_d, cdrb_d, out_d,
          NK, NKT, NKT_CDR, BIAS0, NBT):
    nc = tc.nc
    Exp = mybir.ActivationFunctionType.Exp

    wpool = ctx.enter_context(tc.tile_pool(name="w", bufs=1))
    xpool = ctx.enter_context(tc.tile_pool(name="x", bufs=1))
    qkv = ctx.enter_context(tc.tile_pool(name="qkv", bufs=1))
    psS = ctx.enter_context(tc.tile_pool(name="psS", bufs=2, space="PSUM"))
    psC = ctx.enter_context(tc.tile_pool(name="psC", bufs=2, space="PSUM"))
    psM = ctx.enter_context(tc.tile_pool(name="psM", bufs=2, space="PSUM"))
    pP = ctx.enter_context(tc.tile_pool(name="pP", bufs=2))
    pR = ctx.enter_context(tc.tile_pool(name="pR", bufs=2))
    pO = ctx.enter_context(tc.tile_pool(name="pO", bufs=3))

    # ---- input loads: sync queue feeds k/v path, gpsimd queue the rest ----
    def load4(pool, dram, cols, nm, eng, col_chunks=None):
        """[512, cols] DRAM -> [128, 4*cols] tile; 4 contraction-chunk views."""
        main = pool.tile([128, 4 * cols], BF16, name=f"{nm}m", tag=f"{nm}m")
        mv = main[:].rearrange("p (ch c) -> p ch c", ch=4)
        for n0, ns in (col_chunks or [(0, cols)]):
            eng.dma_start(
                mv[:, :, n0:n0 + ns],
                dram[0:C, n0:n0 + ns].rearrange("(ch p) c -> p ch c", p=128))
        return [main[:, ci * cols:(ci + 1) * cols] for ci in range(4)]

    wks = load4(wpool, wk_d, 256, "wk", nc.sync)
    nkch = _chunks(NK, 512)
    xks_tile = xpool.tile([128, 4 * NK], BF16, name="xkm", tag="xkm")
    xks_v = xks_tile[:].rearrange("p (ch c) -> p ch c", ch=4)
    n0, ns = nkch[0]
    nc.sync.dma_start(xks_v[:, :, n0:n0 + ns],
                      xkT_d[0:C, n0:n0 + ns].rearrange("(ch p) c -> p ch c", p=128))
    wvs = load4(wpool, wv_d, 260, "wv", nc.sync)
    for n0, ns in nkch[1:]:
        nc.sync.dma_start(xks_v[:, :, n0:n0 + ns],
                          xkT_d[0:C, n0:n0 + ns].rearrange("(ch p) c -> p ch c", p=128))
    xks = [xks_tile[:, ci * NK:(ci + 1) * NK] for ci in range(4)]

    # q-path on the gpsimd queue: first x chunk + Wq, then small tensors;
    # the bulky remaining x chunks and Wo are deferred behind the k-path
    xs_tile = xpool.tile([128, 4 * T], BF16, name="xm", tag="xm")
    xs_v = xs_tile[:].rearrange("p (ch c) -> p ch c", ch=4)

    def x_chunk(eng, n0, ns):
        eng.dma_start(xs_v[:, :, n0:n0 + ns],
                      xT_d[0:C, n0:n0 + ns].rearrange("(ch p) c -> p ch c", p=128))

    x_chunk(nc.gpsimd, 0, 512)
    wqs = load4(wpool, wq_d, 256, "wq", nc.gpsimd)
    xs = [xs_tile[:, ci * T:(ci + 1) * T] for ci in range(4)]
    xk_aug = xpool.tile([1, NK], BF16, name="xka", tag="xka")
    nc.gpsimd.dma_start(xk_aug[:], xkT_d[C:C + 1, :])
    wv_aug = wpool.tile([1, 260], BF16, name="wva", tag="wva")
    nc.gpsimd.dma_start(wv_aug[:], wv_d[C:C + 1, :])
    cdrb = wpool.tile([128, max(NBT, 1)], F32, name="cdrb", tag="cdrb")
    nc.gpsimd.dma_start(cdrb[:], cdrb_d[:])
    x_chunk(nc.gpsimd, 512, 512)
    x_chunk(nc.sync, 1024, 512)
    wo_all = wpool.tile([128, 1024], BF16, name="wo", tag="wo")
    nc.gpsimd.dma_start(wo_all[:].rearrange("p (g c) -> p g c", g=2),
                        wo_d[:].rearrange("(g p) c -> p g c", p=128))
    wo = [wo_all[:, 0:512], wo_all[:, 512:1024]]
    x_chunk(nc.sync, 1536, 512)

    # selector matrix for denominator broadcast: E.T @ cu = row 64 of cu
    # replicated across 64 psum partitions
    esel = wpool.tile([65, 64], BF16, name="esel", tag="esel")
    nc.vector.memset(esel[:], 0.0)
    nc.vector.memset(esel[64:65, :], 1.0)

    # ---- persistent activation tiles ------------------------------------
    qT = [qkv.tile([128, T], BF16, name=f"q{p}", tag=f"q{p}") for p in range(2)]
    kT = [qkv.tile([128, NK], BF16, name=f"k{p}", tag=f"k{p}") for p in range(2)]
    v_sb = qkv.tile([128, NKT * 260], BF16, name="v", tag="v")
    ctxn = [qkv.tile([128, T], BF16, name=f"ctxn{p}", tag=f"ctxn{p}")
            for p in range(2)]

    # ---- projection emitters --------------------------------------------
    def k_proj(pp):
        for n0, ns in _chunks(NK, 512):
            mt = psM.tile([128, 512], F32, name="m", tag="m")
            for ci in range(4):
                nc.tensor.matmul(
                    mt[:, :ns],
                    wks[ci][:, pp * 128:(pp + 1) * 128],
                    xks[ci][:, n0:n0 + ns],
                    start=(ci == 0), stop=(ci == 3))
            nc.vector.tensor_copy(kT[pp][:, n0:n0 + ns], mt[:, :ns])

    def v_proj(kt):
        mt = psM.tile([128, 512], F32, name="m", tag="m")
        for ci in range(4):
            nc.tensor.matmul(
                mt[:, 0:260],
                xks[ci][:, kt * 128:(kt + 1) * 128],
                wvs[ci][:],
                start=(ci == 0), stop=False)
        nc.tensor.matmul(
            mt[:, 0:260],
            xk_aug[0:1, kt * 128:(kt + 1) * 128],
            wv_aug[:],
            start=False, stop=True)
        nc.vector.tensor_copy(v_sb[:, kt * 260:(kt + 1) * 260], mt[:, 0:260])

    def q_proj(qc, pp):
        mt = psM.tile([128, 512], F32, name="m", tag="m")
        for ci in range(4):
            nc.tensor.matmul(
                mt[:],
                wqs[ci][:, pp * 128:(pp + 1) * 128],
                xs[ci][:, qc * 512:(qc + 1) * 512],
                start=(ci == 0), stop=(ci == 3))
        nc.vector.tensor_copy(qT[pp][:, qc * 512:(qc + 1) * 512], mt[:])

    def out_proj(qc, tqs):
        for tq in tqs:
            t0 = qc * 512 + tq * 128
            cp = psM.tile([128, 512], F32, name="m", tag="m")
            nc.tensor.matmul(cp[:], ctxn[0][:, t0:t0 + 128], wo[0],
                             start=True, stop=False)
            nc.tensor.matmul(cp[:], ctxn[1][:, t0:t0 + 128], wo[1],
                             start=False, stop=True)
            ot = pO.tile([128, 512], BF16, name="ot", tag="o")
            nc.vector.tensor_copy(ot[:], cp[:])
            nc.sync.dma_start(out_d[t0:t0 + 128, :], ot[:])

    # ---- attention ------------------------------------------------------
    # Split into a scores+exp pass (A) and a ctx+normalize pass (B). A(i)
    # and B(i-1) are emitted interleaved: while phase i's exp activates
    # pace the scalar engine, the previous phase's ctx matmuls (dep-free,
    # P is already in SBUF) keep the PE array dense.
    # P-slot map. pair1 (and pair0's dual region kt<BIAS0): slot 2kt+h so a
    # single [128,1024] act covers both heads of one ktile. pair0's tail:
    # h0-biased singles get slots [2*BIAS0, 2*BIAS0+NBT); h1 tiles kt>=BIAS0
    # are packed CONSECUTIVELY so two ktiles share one act.
    def pslot(pair, kt, h):
        if pair == 1 or kt < BIAS0:
            return 2 * kt + h
        if h == 0:
            return 2 * BIAS0 + (kt - BIAS0)
        return 2 * BIAS0 + NBT + (kt - BIAS0)

    def attn_scores(qc, pair, Pt):
        q0 = qc * 512

        def score_mm(sp, half, pair_, kt, h):
            r0 = h * 64
            nc.tensor.matmul(
                sp[:, half * 512:(half + 1) * 512],
                kT[pair_][r0:r0 + 64, kt * 128:(kt + 1) * 128],
                qT[pair_][r0:r0 + 64, q0:q0 + 512],
                start=True, stop=True,
                tile_position=(r0, 0))

        dual_end = NKT if pair == 1 else BIAS0
        for kt in range(dual_end):
            sp = psS.tile([128, 1024], F32, name="S", tag="S")
            score_mm(sp, 0, pair, kt, 0)
            score_mm(sp, 1, pair, kt, 1)
            nc.scalar.activation(
                Pt[:, pslot(pair, kt, 0) * 512:(pslot(pair, kt, 1) + 1) * 512],
                sp[:], Exp, scale=EXP_SCALE)
            yield
        if pair == 0:
            # h0 biased singles
            for kt in range(BIAS0, NKT_CDR):
                sp = psS.tile([128, 1024], F32, name="S", tag="S")
                score_mm(sp, 0, 0, kt, 0)
                s0 = pslot(0, kt, 0)
                nc.scalar.activation(
                    Pt[:, s0 * 512:(s0 + 1) * 512], sp[:, 0:512],
                    Exp, bias=cdrb[:, kt - BIAS0:kt - BIAS0 + 1],
                    scale=EXP_SCALE)
                yield
            # h1 tail tiles, two ktiles per act
            for kt0 in range(BIAS0, NKT, 2):
                kts = [kt for kt in (kt0, kt0 + 1) if kt < NKT]
                sp = psS.tile([128, 1024], F32, name="S", tag="S")
                for j, kt in enumerate(kts):
                    score_mm(sp, j, 0, kt, 1)
                s0 = pslot(0, kts[0], 1)
                nc.scalar.activation(
                    Pt[:, s0 * 512:(s0 + len(kts)) * 512],
                    sp[:, 0:len(kts) * 512], Exp, scale=EXP_SCALE)
                yield

    def attn_ctx(qc, pair, Pt):
        q0 = qc * 512
        nts = [NKT_CDR if pair == 0 else NKT, NKT]
        cp = [psC.tile([65, 512], F32, name=f"c{h}", tag="ctx")
              for h in range(2)]
        done = [0, 0]
        for kt in range(NKT):
            hs = [h for h in range(2) if kt < nts[h]]
            for h in hs:
                head = 2 * pair + h
                s0 = pslot(pair, kt, h)
                nc.tensor.matmul(
                    cp[h][:],
                    v_sb[:, kt * 260 + head * 65:kt * 260 + (head + 1) * 65],
                    Pt[:, s0 * 512:(s0 + 1) * 512],
                    start=(done[h] == 0), stop=(done[h] + 1 == nts[h]))
                done[h] += 1
                yield
                if done[h] == nts[h]:
                    # normalize: PE-broadcast the denominator row, then DVE
                    # fast-reciprocal + multiply at 64-partition width
                    r0 = h * 64
                    cu = pR.tile([65, 512], BF16, name="cu", tag="cu")
                    nc.vector.tensor_copy(cu[:], cp[h][:])
                    dn = psM.tile([128, 512], F32, name="m", tag="m")
                    nc.tensor.matmul(dn[0:64, :], esel[:], cu[:],
                                     start=True, stop=True)
                    rb = pR.tile([64, 512], F32, name="rb", tag="rb")
                    nc.vector.reciprocal_approx_fast(rb[:], dn[0:64, :])
                    nc.vector.tensor_mul(
                        ctxn[pair][r0:r0 + 64, q0:q0 + 512],
                        cu[0:64, :], rb[:])
                    yield

    # ---- emission order -------------------------------------------------
    # Instruction order per engine is STATIC: a DMA-blocked matmul blocks
    # everything emitted after it on the PE. The emission order below is
    # hand-matched to DMA arrival.
    # PE warmup: one big dummy-matmul block warms the HAM clock gate while
    # the first input DMAs land.
    wps = psM.tile([128, 512], F32, name="m", tag="m")
    for _ in range(150):
        nc.tensor.matmul(wps[0:64, 0:64], esel[:], esel[:],
                         start=True, stop=True)
    nc.vector.tensor_copy(pR.tile([64, 64], F32, name="wd", tag="wd")[:],
                          wps[0:64, 0:64])

    def take(gen, n):
        for _ in range(n):
            if next(gen, StopIteration) is StopIteration:
                return False
        return True

    # pair 1 before pair 0 within each qc: the final phase is then the CDR
    # pair whose h0 stream ends early, shrinking the serial tail
    phases = [(qc, pair) for qc in range(4) for pair in (1, 0)]
    NPH = len(phases)
    Bs = [None] * NPH
    drained = [0] * NPH

    def mk_phase(i):
        qc, pair = phases[i]
        Pt = pP.tile([128, NKT * 1024], BF16, name="P", tag="P")
        Bs[i] = attn_ctx(qc, pair, Pt)
        return attn_scores(qc, pair, Pt)

    def drain_one(j, step_cap=None):
        if Bs[j] is None:
            return False
        if step_cap is not None and drained[j] >= step_cap:
            return False
        if next(Bs[j], StopIteration) is StopIteration:
            Bs[j] = None
            return False
        drained[j] += 1
        return True

    # --- prologue, software-pipelined against the k-path DMA chunks ------
    def k_proj_chunk(pp, n0, ns):
        mt = psM.tile([128, 512], F32, name="m", tag="m")
        for ci in range(4):
            nc.tensor.matmul(
                mt[:, :ns],
                wks[ci][:, pp * 128:(pp + 1) * 128],
                xks[ci][:, n0:n0 + ns],
                start=(ci == 0), stop=(ci == 3))
        nc.vector.tensor_copy(kT[pp][:, n0:n0 + ns], mt[:, :ns])

    A0 = mk_phase(0)
    p0 = phases[0][1]
    first = True
    for n0, ns in _chunks(NK, 512):
        k_proj_chunk(p0, n0, ns)
        k_proj_chunk(1 - p0, n0, ns)
        if first:
            q_proj(0, p0)
            q_proj(0, 1 - p0)
            first = False
        take(A0, ns // 128)          # scores for the ktiles this chunk covers
        for kt in range(n0 // 128, (n0 + ns) // 128):
            v_proj(kt)

    # --- main phase loop -------------------------------------------------
    # Per A-step: drain the previous phase's ctx (fine-grained yields), then
    # head-start this phase's own ctx. The last two A-steps emit nothing
    # else (clean runway so the next phase's first scores issue promptly).
    for i in range(1, NPH):
        A = mk_phase(i)
        ny = (BIAS0 + NBT + (NKT - BIAS0 + 1) // 2
              if phases[i][1] == 0 else NKT)
        for step, _ in enumerate(A):
            budget = 0 if step >= ny - 2 else 3
            if i >= 2:
                while budget and drain_one(i - 2):
                    budget -= 1
            while budget and drain_one(i - 1):
                budget -= 1
            while budget and drain_one(i, step_cap=2 * step):
                budget -= 1
            if step == 3 and i + 1 < NPH and phases[i + 1][0] >= 1:
                q_proj(*phases[i + 1])
            if i >= 3 and 1 <= step < 3:
                qd = (i - 3) // 2
                tqs = (0, 1) if (i - 3) % 2 == 0 else (2, 3)
                out_proj(qd, (tqs[step - 1],))
            if i == NPH - 1 and step == 5:
                out_proj(2, (2,))
                out_proj(2, (3,))
        if i >= 2:
            while drain_one(i - 2):
                pass
    for j in (NPH - 2, NPH - 1):
        while drain_one(j):
            pass
    out_proj(3, range(4))


# ---------------------------------------------------------------------------
# host side
# ---------------------------------------------------------------------------

def _host_prep(x, mask, cdrs_score, Wq, bq, Wk, bk, Wv, bv, Wo, bo):
    x = np.ascontiguousarray(np.asarray(x, np.float32))
    mask = np.asarray(mask)
    cdrs = np.asarray(cdrs_score)
    Wq = np.asarray(Wq, np.float32)
    Wk = np.asarray(Wk, np.float32)
    Wv = np.asarray(Wv, np.float32)
    Wo = np.asarray(Wo, np.float32)
    bv = np.asarray(bv, np.float32)
    assert np.abs(np.asarray(bq)).max() < 1e-6, "nonzero bq unsupported"
    assert np.abs(np.asarray(bk)).max() < 1e-6, "nonzero bk unsupported"

    gathers = []
    for b in range(B):
        valid = mask[b] == 1
        cdrv = valid & (cdrs[b] == 1) if np.any(cdrs[b] == 1) else valid
        regv = valid & ~cdrv
        gathers.append((np.nonzero(cdrv)[0], np.nonzero(regv)[0]))
    ncdrs = [len(g[0]) for g in gathers]
    valids = [len(g[0]) + len(g[1]) for g in gathers]
    NKT = max(1, math.ceil(max(valids) / 128))
    NK = NKT * 128
    NKT_CDR = max(1, math.ceil(max(ncdrs) / 128))
    BIAS0 = min(ncdrs) // 128
    NBT = NKT_CDR - BIAS0

    # per-group weight bundles (shared across samples)
    wbund = []
    for g in range(2):
        heads = [g, g + 2, g + 4, g + 6]
        dims = np.concatenate([np.arange(h * D, (h + 1) * D) for h in heads])
        wq_c = Wq[:, dims]
        wk_c = Wk[:, dims]
        wv_cols = []
        for h in heads:
            hd = np.arange(h * D, (h + 1) * D)
            wv = np.concatenate([Wv[:, hd], bv[hd][None, :]], axis=0)
            sel = np.zeros((C + 1, 1), np.float32)
            sel[C, 0] = 1.0
            wv_cols.append(np.concatenate([wv, sel], axis=1))
        wv_aug = np.concatenate(wv_cols, axis=1)
        wo_rows = Wo[dims, :]
        wbund.append(tuple(
            np.ascontiguousarray(w.astype(ml_dtypes.bfloat16))
            for w in (wq_c, wk_c, wv_aug, wo_rows)))

    in_maps = []
    for b in range(B):
        idx_cdr, idx_reg = gathers[b]
        nv = len(idx_cdr) + len(idx_reg)
        xk = np.zeros((NK, C), np.float32)
        xk[:len(idx_cdr)] = x[b, idx_cdr]
        xk[len(idx_cdr):nv] = x[b, idx_reg]
        ones_row = np.zeros((1, NK), np.float32)
        ones_row[0, :nv] = 1.0
        xkT_aug = np.ascontiguousarray(
            np.concatenate([xk.T, ones_row], axis=0))
        xT_bf = np.ascontiguousarray(x[b].T.astype(ml_dtypes.bfloat16))
        xkT_bf = np.ascontiguousarray(xkT_aug.astype(ml_dtypes.bfloat16))
        cdrb = np.zeros((128, max(NBT, 1)), np.float32)
        for t in range(NBT):
            keys = (BIAS0 + t) * 128 + np.arange(128)
            cdrb[:, t] = np.where(keys < len(idx_cdr), 0.0, MASK_BIAS)
        for g in range(2):
            wq_c, wk_c, wv_aug, wo_rows = wbund[g]
            in_maps.append({
                "xT": xT_bf, "xkT": xkT_bf,
                "Wq": wq_c, "Wk": wk_c, "Wv": wv_aug, "Wo": wo_rows,
                "cdrb": cdrb,
            })
    return in_maps, NKT, NKT_CDR, BIAS0, NBT


def kernel(**inputs) -> np.ndarray:
    global LAST_RESULTS
    in_maps, NKT, NKT_CDR, BIAS0, NBT = _host_prep(**inputs)

    key = (NKT, NKT_CDR, BIAS0, NBT)
    nc = _PROGRAM_CACHE.get(key)
    if nc is None:
        nc = _build_program(NKT, NKT_CDR, BIAS0, NBT)
        _PROGRAM_CACHE[key] = nc

    res = run_bass_kernel_spmd(nc, in_maps, core_ids=list(range(8)))
    LAST_RESULTS = res

    bo = np.asarray(inputs["bo"], np.float32)
    out = np.empty((B, T, C), np.float32)
    for b in range(B):
        out[b] = (np.asarray(res.results[2 * b]["out"], np.float32)
                  + np.asarray(res.results[2 * b + 1]["out"], np.float32)
                  + bo[None, :])
    return out


# revision 37
# speedup vs baseline: 1.1698x; 1.1698x over previous
"""Trainium2 Bass kernel for nn_CDRsAttention (sparse multi-head attention
with padding mask + CDR key mask on the first 2 heads).

Sharding: 8 cores = 4 samples (B) x 2 head groups. Core (b, g) computes
heads [g, g+2, g+4, g+6] of sample b (exactly one CDR head each), producing
a partial output ctx_heads @ Wo_rows; the host sums the two partials + bo.

Host-side prep (pure numpy, cheap):
  - per-sample key gather: only keys with mask==1 participate, CDR-valid
    keys first, then regular keys, zero-padded to NK = 128*ceil(max valid).
    No inter-region padding: the CDR head attends tiles [0, NKT_CDR) and
    masks intruding regular keys via a per-partition additive bias (-30)
    fed to the exp activation for the boundary tiles.
  - xkT carries one extra row (valid-key indicator) that flows through
    augmented Wv selector columns so v column h*65+64 is the indicator row,
    making ctx^T row 64 the softmax denominator (padded keys drop out).
  - q/k biases are zero, so their projections contract over exactly 512
    rows (no augmented row).

Device per core (bf16 matmuls):
  qT/kT/v projections -> per head: S^T = kT_tile^T @ qT (keys on psum
  partitions), P = exp(S^T/8) on ScalarE straight out of PSUM (pairs of
  ktiles per activate), ctx^T accumulated as v_aug^T @ P, then a
  fast-reciprocal of the denominator row, a PE broadcast matmul (f32r),
  a DVE normalization multiply, and the output projection
  out = ctx_norm^T.T @ Wo_rows streamed straight to DRAM per 128-query
  slab. q/out projections are interleaved as PE filler between attention
  groups to keep the PE array dense and HAM-warm.
"""
import math
from contextlib import ExitStack

import ml_dtypes
import numpy as np

import concourse.bass as bass
import concourse.mybir as mybir
import concourse.tile as tile
from concourse import bacc
from concourse.bass_utils import run_bass_kernel_spmd

B, T, C, H, D = 4, 2048, 512, 8, 64
F32 = mybir.dt.float32
F32R = mybir.dt.float32r
BF16 = mybir.dt.bfloat16
EXP_SCALE = 1.0 / 8.0  # 1/sqrt(D)
MASK_BIAS = -30.0

_PROGRAM_CACHE: dict = {}
LAST_RESULTS = None  # BassKernelResults of the most recent kernel() call


def _chunks(total, step):
    return [(i, min(step, total - i)) for i in range(0, total, step)]


def _build_program(NKT, NKT_CDR, BIAS0, NBT):
    NK = NKT * 128
    nc = bacc.Bacc("TRN2", target_bir_lowering=False, debug=False, num_devices=8)
    xT_d = nc.dram_tensor("xT", [C, T], BF16, kind="ExternalInput").ap()
    xkT_d = nc.dram_tensor("xkT", [C + 1, NK], BF16, kind="ExternalInput").ap()
    wq_d = nc.dram_tensor("Wq", [C, 256], BF16, kind="ExternalInput").ap()
    wk_d = nc.dram_tensor("Wk", [C, 256], BF16, kind="ExternalInput").ap()
    wv_d = nc.dram_tensor("Wv", [C + 1, 260], BF16, kind="ExternalInput").ap()
    wo_d = nc.dram_tensor("Wo", [256, 512], BF16, kind="ExternalInput").ap()
    cdrb_d = nc.dram_tensor("cdrb", [128, max(NBT, 1)], F32,
                            kind="ExternalInput").ap()
    out_d = nc.dram_tensor("out", [T, 512], F32, kind="ExternalOutput").ap()

    with tile.TileContext(nc) as tc:
        with ExitStack() as ctx:
            _body(ctx, tc, xT_d, xkT_d, wq_d, wk_d, wv_d, wo_d, cdrb_d, out_d,
                  NK, NKT, NKT_CDR, BIAS0, NBT)
    nc.compile()
    return nc


def _body(ctx, tc, xT_d, xkT_d, wq_d, wk_d, wv_d, wo_d, cdrb_d, out_d,
          NK, NKT, NKT_CDR, BIAS0, NBT):
    nc = tc.nc
    Exp = mybir.ActivationFunctionType.Exp

    wpool = ctx.enter_context(tc.tile_pool(name="w", bufs=1))
    xpool = ctx.enter_context(tc.tile_pool(name="x", bufs=1))
    qkv = ctx.enter_context(tc.tile_pool(name="qkv", bufs=1))
    psS = ctx.enter_context(tc.tile_pool(name="psS", bufs=3, space="PSUM"))
    psC = ctx.enter_context(tc.tile_pool(name="psC", bufs=2, space="PSUM"))
    pP = ctx.enter_context(tc.tile_pool(name="pP", bufs=2))
    pR = ctx.enter_context(tc.tile_pool(name="pR", bufs=2))
    pO = ctx.enter_context(tc.tile_pool(name="pO", bufs=3))

    # ---- input loads: sync queue feeds k/v path, gpsimd queue the rest ----
    def load4(pool, dram, cols, nm, eng, col_chunks=None):
        """[512, cols] DRAM -> [128, 4*cols] tile; 4 contraction-chunk views."""
        main = pool.tile([128, 4 * cols], BF16, name=f"{nm}m", tag=f"{nm}m")
        mv = main[:].rearrange("p (ch c) -> p ch c", ch=4)
        for n0, ns in (col_chunks or [(0, cols)]):
            eng.dma_start(
                mv[:, :, n0:n0 + ns],
                dram[0:C, n0:n0 + ns].rearrange("(ch p) c -> p ch c", p=128))
        return [main[:, ci * cols:(ci + 1) * cols] for ci in range(4)]

    wks = load4(wpool, wk_d, 256, "wk", nc.sync)
    nkch = _chunks(NK, 512)
    xks_tile = xpool.tile([128, 4 * NK], BF16, name="xkm", tag="xkm")
    xks_v = xks_tile[:].rearrange("p (ch c) -> p ch c", ch=4)
    n0, ns = nkch[0]
    nc.sync.dma_start(xks_v[:, :, n0:n0 + ns],
                      xkT_d[0:C, n0:n0 + ns].rearrange("(ch p) c -> p ch c", p=128))
    wvs = load4(wpool, wv_d, 260, "wv", nc.sync)
    for n0, ns in nkch[1:]:
        nc.sync.dma_start(xks_v[:, :, n0:n0 + ns],
                          xkT_d[0:C, n0:n0 + ns].rearrange("(ch p) c -> p ch c", p=128))
    xks = [xks_tile[:, ci * NK:(ci + 1) * NK] for ci in range(4)]

    # q-path on the gpsimd queue: first x chunk + Wq, then small tensors;
    # the bulky remaining x chunks and Wo are deferred behind the k-path
    xs_tile = xpool.tile([128, 4 * T], BF16, name="xm", tag="xm")
    xs_v = xs_tile[:].rearrange("p (ch c) -> p ch c", ch=4)

    def x_chunk(eng, n0, ns):
        eng.dma_start(xs_v[:, :, n0:n0 + ns],
                      xT_d[0:C, n0:n0 + ns].rearrange("(ch p) c -> p ch c", p=128))

    x_chunk(nc.gpsimd, 0, 512)
    wqs = load4(wpool, wq_d, 256, "wq", nc.gpsimd)
    xs = [xs_tile[:, ci * T:(ci + 1) * T] for ci in range(4)]
    xk_aug = xpool.tile([1, NK], BF16, name="xka", tag="xka")
    nc.gpsimd.dma_start(xk_aug[:], xkT_d[C:C + 1, :])
    wv_aug = wpool.tile([1, 260], BF16, name="wva", tag="wva")
    nc.gpsimd.dma_start(wv_aug[:], wv_d[C:C + 1, :])
    cdrb = wpool.tile([128, max(NBT, 1)], F32, name="cdrb", tag="cdrb")
    nc.gpsimd.dma_start(cdrb[:], cdrb_d[:])
    x_chunk(nc.gpsimd, 512, 512)
    x_chunk(nc.sync, 1024, 512)
    wo_all = wpool.tile([128, 1024], BF16, name="wo", tag="wo")
    nc.gpsimd.dma_start(wo_all[:].rearrange("p (g c) -> p g c", g=2),
                        wo_d[:].rearrange("(g p) c -> p g c", p=128))
    wo = [wo_all[:, 0:512], wo_all[:, 512:1024]]
    x_chunk(nc.sync, 1536, 512)

    # selector matrix for denominator broadcast: E.T @ cu = row 64 of cu
    # replicated across 64 psum partitions
    esel = wpool.tile([65, 64], BF16, name="esel", tag="esel")
    nc.vector.memset(esel[:], 0.0)
    nc.vector.memset(esel[64:65, :], 1.0)

    # ---- persistent activation tiles ------------------------------------
    qT = [qkv.tile([128, T], BF16, name=f"q{p}", tag=f"q{p}") for p in range(2)]
    kT = [qkv.tile([128, NK], BF16, name=f"k{p}", tag=f"k{p}") for p in range(2)]
    v_sb = qkv.tile([128, NKT * 260], BF16, name="v", tag="v")
    ctxn = [qkv.tile([128, T], BF16, name=f"ctxn{p}", tag=f"ctxn{p}")
            for p in range(2)]

    # ---- projection emitters --------------------------------------------
    def k_proj(pp):
        for n0, ns in _chunks(NK, 512):
            mt = psS.tile([128, 1024], F32, name="S", tag="S")
            for ci in range(4):
                nc.tensor.matmul(
                    mt[:, :ns],
                    wks[ci][:, pp * 128:(pp + 1) * 128],
                    xks[ci][:, n0:n0 + ns],
                    start=(ci == 0), stop=(ci == 3))
            nc.vector.tensor_copy(kT[pp][:, n0:n0 + ns], mt[:, :ns])

    def v_proj(kt):
        mt = psS.tile([128, 1024], F32, name="S", tag="S")
        for ci in range(4):
            nc.tensor.matmul(
                mt[:, 0:260],
                xks[ci][:, kt * 128:(kt + 1) * 128],
                wvs[ci][:],
                start=(ci == 0), stop=False)
        nc.tensor.matmul(
            mt[:, 0:260],
            xk_aug[0:1, kt * 128:(kt + 1) * 128],
            wv_aug[:],
            start=False, stop=True)
        nc.vector.tensor_copy(v_sb[:, kt * 260:(kt + 1) * 260], mt[:, 0:260])

    def q_proj(qc, pp):
        mt = psS.tile([128, 1024], F32, name="S", tag="S")
        for ci in range(4):
            nc.tensor.matmul(
                mt[:, 0:512],
                wqs[ci][:, pp * 128:(pp + 1) * 128],
                xs[ci][:, qc * 512:(qc + 1) * 512],
                start=(ci == 0), stop=(ci == 3))
        nc.vector.tensor_copy(qT[pp][:, qc * 512:(qc + 1) * 512], mt[:, 0:512])

    def out_proj(qc, tqs):
        for tq in tqs:
            t0 = qc * 512 + tq * 128
            cp = psS.tile([128, 1024], F32, name="S", tag="S")
            nc.tensor.matmul(cp[:, 0:512], ctxn[0][:, t0:t0 + 128], wo[0],
                             start=True, stop=False)
            nc.tensor.matmul(cp[:, 0:512], ctxn[1][:, t0:t0 + 128], wo[1],
                             start=False, stop=True)
            ot = pO.tile([128, 512], F32, name="ot", tag="o")
            nc.vector.tensor_copy(ot[:], cp[:, 0:512])
            nc.sync.dma_start(out_d[t0:t0 + 128, :], ot[:])

    # ---- attention ------------------------------------------------------
    # Split into a scores+exp pass (A) and a ctx+normalize pass (B). A(i)
    # and B(i-1) are emitted interleaved: while phase i's exp activates
    # pace the scalar engine, the previous phase's ctx matmuls (dep-free,
    # P is already in SBUF) keep the PE array dense.
    # P-slot map. pair1 (and pair0's dual region kt<BIAS0): slot 2kt+h so a
    # single [128,1024] act covers both heads of one ktile. pair0's tail:
    # h0-biased singles get slots [2*BIAS0, 2*BIAS0+NBT); h1 tiles kt>=BIAS0
    # are packed CONSECUTIVELY so two ktiles share one act.
    def pslot(pair, kt, h):
        if pair == 1 or kt < BIAS0:
            return 2 * kt + h
        if h == 0:
            return 2 * BIAS0 + (kt - BIAS0)
        return 2 * BIAS0 + NBT + (kt - BIAS0)

    def attn_scores(qc, pair, Pt):
        q0 = qc * 512

        def score_mm(sp, half, pair_, kt, h):
            r0 = h * 64
            nc.tensor.matmul(
                sp[:, half * 512:(half + 1) * 512],
                kT[pair_][r0:r0 + 64, kt * 128:(kt + 1) * 128],
                qT[pair_][r0:r0 + 64, q0:q0 + 512],
                start=True, stop=True,
                tile_position=(r0, 0))

        dual_end = NKT if pair == 1 else BIAS0
        for kt in range(dual_end):
            sp = psS.tile([128, 1024], F32, name="S", tag="S")
            score_mm(sp, 0, pair, kt, 0)
            score_mm(sp, 1, pair, kt, 1)
            nc.scalar.activation(
                Pt[:, pslot(pair, kt, 0) * 512:(pslot(pair, kt, 1) + 1) * 512],
                sp[:], Exp, scale=EXP_SCALE)
            yield
        if pair == 0:
            # h0 biased singles
            for kt in range(BIAS0, NKT_CDR):
                sp = psS.tile([128, 1024], F32, name="S", tag="S")
                score_mm(sp, 0, 0, kt, 0)
                s0 = pslot(0, kt, 0)
                nc.scalar.activation(
                    Pt[:, s0 * 512:(s0 + 1) * 512], sp[:, 0:512],
                    Exp, bias=cdrb[:, kt - BIAS0:kt - BIAS0 + 1],
                    scale=EXP_SCALE)
                yield
            # h1 tail tiles, two ktiles per act
            for kt0 in range(BIAS0, NKT, 2):
                kts = [kt for kt in (kt0, kt0 + 1) if kt < NKT]
                sp = psS.tile([128, 1024], F32, name="S", tag="S")
                for j, kt in enumerate(kts):
                    score_mm(sp, j, 0, kt, 1)
                s0 = pslot(0, kts[0], 1)
                nc.scalar.activation(
                    Pt[:, s0 * 512:(s0 + len(kts)) * 512],
                    sp[:, 0:len(kts) * 512], Exp, scale=EXP_SCALE)
                yield

    def attn_ctx(qc, pair, Pt):
        q0 = qc * 512
        nts = [NKT_CDR if pair == 0 else NKT, NKT]
        cp = [psC.tile([65, 512], F32, name=f"c{h}", tag="ctx")
              for h in range(2)]
        done = [0, 0]
        for kt in range(NKT):
            hs = [h for h in range(2) if kt < nts[h]]
            for h in hs:
                head = 2 * pair + h
                s0 = pslot(pair, kt, h)
                nc.tensor.matmul(
                    cp[h][:],
                    v_sb[:, kt * 260 + head * 65:kt * 260 + (head + 1) * 65],
                    Pt[:, s0 * 512:(s0 + 1) * 512],
                    start=(done[h] == 0), stop=(done[h] + 1 == nts[h]))
                done[h] += 1
                yield
                if done[h] == nts[h]:
                    # normalize: PE-broadcast the denominator row, then DVE
                    # fast-reciprocal + multiply at 64-partition width
                    r0 = h * 64
                    cu = pR.tile([65, 512], BF16, name="cu", tag="cu")
                    nc.vector.tensor_copy(cu[:], cp[h][:])
                    dn = psC.tile([65, 512], F32, name="dnc", tag="ctx")
                    nc.tensor.matmul(dn[0:64, :], esel[:], cu[:],
                                     start=True, stop=True)
                    rb = pR.tile([64, 512], F32, name="rb", tag="rb")
                    nc.vector.reciprocal_approx_fast(rb[:], dn[0:64, :])
                    nc.vector.tensor_mul(
                        ctxn[pair][r0:r0 + 64, q0:q0 + 512],
                        cu[0:64, :], rb[:])
                    yield

    # ---- emission order -------------------------------------------------
    # Instruction order per engine is STATIC: a DMA-blocked matmul blocks
    # everything emitted after it on the PE. The emission order below is
    # hand-matched to DMA arrival.
    # PE warmup: one big dummy-matmul block warms the HAM clock gate while
    # the first input DMAs land.
    wps = psS.tile([128, 1024], F32, name="S", tag="S")
    for _ in range(150):
        nc.tensor.matmul(wps[0:64, 0:64], esel[:], esel[:],
                         start=True, stop=True)
    nc.vector.tensor_copy(pR.tile([64, 64], F32, name="wd", tag="wd")[:],
                          wps[0:64, 0:64])

    def take(gen, n):
        for _ in range(n):
            if next(gen, StopIteration) is StopIteration:
                return False
        return True

    # pair 1 before pair 0 within each qc: the final phase is then the CDR
    # pair whose h0 stream ends early, shrinking the serial tail
    phases = [(qc, pair) for qc in range(4) for pair in (1, 0)]
    NPH = len(phases)
    Bs = [None] * NPH
    drained = [0] * NPH

    def mk_phase(i):
        qc, pair = phases[i]
        Pt = pP.tile([128, NKT * 1024], BF16, name="P", tag="P")
        Bs[i] = attn_ctx(qc, pair, Pt)
        return attn_scores(qc, pair, Pt)

    def drain_one(j, step_cap=None):
        if Bs[j] is None:
            return False
        if step_cap is not None and drained[j] >= step_cap:
            return False
        if next(Bs[j], StopIteration) is StopIteration:
            Bs[j] = None
            return False
        drained[j] += 1
        return True

    # --- prologue, software-pipelined against the k-path DMA chunks ------
    def k_proj_chunk(pp, n0, ns):
        mt = psS.tile([128, 1024], F32, name="S", tag="S")
        for ci in range(4):
            nc.tensor.matmul(
                mt[:, :ns],
                wks[ci][:, pp * 128:(pp + 1) * 128],
                xks[ci][:, n0:n0 + ns],
                start=(ci == 0), stop=(ci == 3))
        nc.vector.tensor_copy(kT[pp][:, n0:n0 + ns], mt[:, :ns])

    A0 = mk_phase(0)
    p0 = phases[0][1]
    first = True
    for n0, ns in _chunks(NK, 512):
        k_proj_chunk(p0, n0, ns)
        k_proj_chunk(1 - p0, n0, ns)
        if first:
            q_proj(0, p0)
            q_proj(0, 1 - p0)
            first = False
        take(A0, ns // 128)          # scores for the ktiles this chunk covers
        for kt in range(n0 // 128, (n0 + ns) // 128):
            v_proj(kt)

    # --- main phase loop -------------------------------------------------
    # Per A-step: drain the previous phase's ctx (fine-grained yields), then
    # head-start this phase's own ctx. The last two A-steps emit nothing
    # else (clean runway so the next phase's first scores issue promptly).
    for i in range(1, NPH):
        A = mk_phase(i)
        ny = (BIAS0 + NBT + (NKT - BIAS0 + 1) // 2
              if phases[i][1] == 0 else NKT)
        for step, _ in enumerate(A):
            budget = 0 if step >= ny - 2 else 3
            if i >= 2:
                while budget and drain_one(i - 2):
                    budget -= 1
            while budget and drain_one(i - 1):
                budget -= 1
            while budget and drain_one(i, step_cap=2 * step):
                budget -= 1
            if step == 3 and i + 1 < NPH and phases[i + 1][0] >= 1:
                q_proj(*phases[i + 1])
            if i >= 3 and 1 <= step < 3:
                qd = (i - 3) // 2
                tqs = (0, 1) if (i - 3) % 2 == 0 else (2, 3)
                out_proj(qd, (tqs[step - 1],))
            if i == NPH - 1 and step == 5:
                out_proj(2, (2,))
                out_proj(2, (3,))
        if i >= 2:
            while drain_one(i - 2):
                pass
    for j in (NPH - 2, NPH - 1):
        while drain_one(j):
            pass
    out_proj(3, range(4))


# ---------------------------------------------------------------------------
# host side
# ---------------------------------------------------------------------------

def _host_prep(x, mask, cdrs_score, Wq, bq, Wk, bk, Wv, bv, Wo, bo):
    x = np.ascontiguousarray(np.asarray(x, np.float32))
    mask = np.asarray(mask)
    cdrs = np.asarray(cdrs_score)
    Wq = np.asarray(Wq, np.float32)
    Wk = np.asarray(Wk, np.float32)
    Wv = np.asarray(Wv, np.float32)
    Wo = np.asarray(Wo, np.float32)
    bv = np.asarray(bv, np.float32)
    assert np.abs(np.asarray(bq)).max() < 1e-6, "nonzero bq unsupported"
    assert np.abs(np.asarray(bk)).max() < 1e-6, "nonzero bk unsupported"

    gathers = []
    for b in range(B):
        valid = mask[b] == 1
        cdrv = valid & (cdrs[b] == 1) if np.any(cdrs[b] == 1) else valid
        regv = valid & ~cdrv
        gathers.append((np.nonzero(cdrv)[0], np.nonzero(regv)[0]))
    ncdrs = [len(g[0]) for g in gathers]
    valids = [len(g[0]) + len(g[1]) for g in gathers]
    NKT = max(1, math.ceil(max(valids) / 128))
    NK = NKT * 128
    NKT_CDR = max(1, math.ceil(max(ncdrs) / 128))
    BIAS0 = min(ncdrs) // 128
    NBT = NKT_CDR - BIAS0

    # per-group weight bundles (shared across samples)
    wbund = []
    for g in range(2):
        heads = [g, g + 2, g + 4, g + 6]
        dims = np.concatenate([np.arange(h * D, (h + 1) * D) for h in heads])
        wq_c = Wq[:, dims]
        wk_c = Wk[:, dims]
        wv_cols = []
        for h in heads:
            hd = np.arange(h * D, (h + 1) * D)
            wv = np.concatenate([Wv[:, hd], bv[hd][None, :]], axis=0)
            sel = np.zeros((C + 1, 1), np.float32)
            sel[C, 0] = 1.0
            wv_cols.append(np.concatenate([wv, sel], axis=1))
        wv_aug = np.concatenate(wv_cols, axis=1)
        wo_rows = Wo[dims, :]
        wbund.append(tuple(
            np.ascontiguousarray(w.astype(ml_dtypes.bfloat16))
            for w in (wq_c, wk_c, wv_aug, wo_rows)))

    in_maps = []
    for b in range(B):
        idx_cdr, idx_reg = gathers[b]
        nv = len(idx_cdr) + len(idx_reg)
        xk = np.zeros((NK, C), np.float32)
        xk[:len(idx_cdr)] = x[b, idx_cdr]
        xk[len(idx_cdr):nv] = x[b, idx_reg]
        ones_row = np.zeros((1, NK), np.float32)
        ones_row[0, :nv] = 1.0
        xkT_aug = np.ascontiguousarray(
            np.concatenate([xk.T, ones_row], axis=0))
        xT_bf = np.ascontiguousarray(x[b].T.astype(ml_dtypes.bfloat16))
        xkT_bf = np.ascontiguousarray(xkT_aug.astype(ml_dtypes.bfloat16))
        cdrb = np.zeros((128, max(NBT, 1)), np.float32)
        for t in range(NBT):
            keys = (BIAS0 + t) * 128 + np.arange(128)
            cdrb[:, t] = np.where(keys < len(idx_cdr), 0.0, MASK_BIAS)
        for g in range(2):
            wq_c, wk_c, wv_aug, wo_rows = wbund[g]
            in_maps.append({
                "xT": xT_bf, "xkT": xkT_bf,
                "Wq": wq_c, "Wk": wk_c, "Wv": wv_aug, "Wo": wo_rows,
                "cdrb": cdrb,
            })
    return in_maps, NKT, NKT_CDR, BIAS0, NBT


def kernel(**inputs) -> np.ndarray:
    global LAST_RESULTS
    in_maps, NKT, NKT_CDR, BIAS0, NBT = _host_prep(**inputs)

    key = (NKT, NKT_CDR, BIAS0, NBT)
    nc = _PROGRAM_CACHE.get(key)
    if nc is None:
        nc = _build_program(NKT, NKT_CDR, BIAS0, NBT)
        _PROGRAM_CACHE[key] = nc

    res = run_bass_kernel_spmd(nc, in_maps, core_ids=list(range(8)))
    LAST_RESULTS = res

    bo = np.asarray(inputs["bo"], np.float32)
    out = np.empty((B, T, C), np.float32)
    for b in range(B):
        out[b] = res.results[2 * b]["out"] + res.results[2 * b + 1]["out"] + bo[None, :]
    return out


# revision 39
# speedup vs baseline: 1.1741x; 1.0037x over previous
"""Trainium2 Bass kernel for nn_CDRsAttention (sparse multi-head attention
with padding mask + CDR key mask on the first 2 heads).

Sharding: 8 cores = 4 samples (B) x 2 head groups. Core (b, g) computes
heads [g, g+2, g+4, g+6] of sample b (exactly one CDR head each), producing
a partial output ctx_heads @ Wo_rows; the host sums the two partials + bo.

Host-side prep (pure numpy, cheap):
  - per-sample key gather: only keys with mask==1 participate, CDR-valid
    keys first, then regular keys, zero-padded to NK = 128*ceil(max valid).
    No inter-region padding: the CDR head attends tiles [0, NKT_CDR) and
    masks intruding regular keys via a per-partition additive bias (-30)
    fed to the exp activation for the boundary tiles.
  - xkT carries one extra row (valid-key indicator) that flows through
    augmented Wv selector columns so v column h*65+64 is the indicator row,
    making ctx^T row 64 the softmax denominator (padded keys drop out).
  - q/k biases are zero, so their projections contract over exactly 512
    rows (no augmented row).

Device per core (bf16 matmuls):
  qT/kT/v projections -> per head: S^T = kT_tile^T @ qT (keys on psum
  partitions), P = exp(S^T/8) on ScalarE straight out of PSUM (pairs of
  ktiles per activate), ctx^T accumulated as v_aug^T @ P, then a
  fast-reciprocal of the denominator row, a PE broadcast matmul (f32r),
  a DVE normalization multiply, and the output projection
  out = ctx_norm^T.T @ Wo_rows streamed straight to DRAM per 128-query
  slab. q/out projections are interleaved as PE filler between attention
  groups to keep the PE array dense and HAM-warm.
"""
import math
from contextlib import ExitStack

import ml_dtypes
import numpy as np

import concourse.bass as bass
import concourse.mybir as mybir
import concourse.tile as tile
from concourse import bacc
from concourse.bass_utils import run_bass_kernel_spmd

B, T, C, H, D = 4, 2048, 512, 8, 64
F32 = mybir.dt.float32
F32R = mybir.dt.float32r
BF16 = mybir.dt.bfloat16
EXP_SCALE = 1.0 / 8.0  # 1/sqrt(D)
MASK_BIAS = -30.0

_PROGRAM_CACHE: dict = {}
LAST_RESULTS = None  # BassKernelResults of the most recent kernel() call


def _chunks(total, step):
    return [(i, min(step, total - i)) for i in range(0, total, step)]


def _build_program(NKT, NKT_CDR, BIAS0, NBT):
    NK = NKT * 128
    nc = bacc.Bacc("TRN2", target_bir_lowering=False, debug=False, num_devices=8)
    xT_d = nc.dram_tensor("xT", [C, T], BF16, kind="ExternalInput").ap()
    xkT_d = nc.dram_tensor("xkT", [C + 1, NK], BF16, kind="ExternalInput").ap()
    wq_d = nc.dram_tensor("Wq", [C, 256], BF16, kind="ExternalInput").ap()
    wk_d = nc.dram_tensor("Wk", [C, 256], BF16, kind="ExternalInput").ap()
    wv_d = nc.dram_tensor("Wv", [C + 1, 260], BF16, kind="ExternalInput").ap()
    wo_d = nc.dram_tensor("Wo", [256, 512], BF16, kind="ExternalInput").ap()
    cdrb_d = nc.dram_tensor("cdrb", [128, max(NBT, 1)], F32,
                            kind="ExternalInput").ap()
    out_d = nc.dram_tensor("out", [T, 512], F32, kind="ExternalOutput").ap()

    with tile.TileContext(nc) as tc:
        with ExitStack() as ctx:
            _body(ctx, tc, xT_d, xkT_d, wq_d, wk_d, wv_d, wo_d, cdrb_d, out_d,
                  NK, NKT, NKT_CDR, BIAS0, NBT)
    nc.compile()
    return nc


def _body(ctx, tc, xT_d, xkT_d, wq_d, wk_d, wv_d, wo_d, cdrb_d, out_d,
          NK, NKT, NKT_CDR, BIAS0, NBT):
    nc = tc.nc
    Exp = mybir.ActivationFunctionType.Exp

    wpool = ctx.enter_context(tc.tile_pool(name="w", bufs=1))
    xpool = ctx.enter_context(tc.tile_pool(name="x", bufs=1))
    qkv = ctx.enter_context(tc.tile_pool(name="qkv", bufs=1))
    psS = ctx.enter_context(tc.tile_pool(name="psS", bufs=2, space="PSUM"))
    psC = ctx.enter_context(tc.tile_pool(name="psC", bufs=2, space="PSUM"))
    psM = ctx.enter_context(tc.tile_pool(name="psM", bufs=2, space="PSUM"))
    pP = ctx.enter_context(tc.tile_pool(name="pP", bufs=2))
    pR = ctx.enter_context(tc.tile_pool(name="pR", bufs=2))
    pO = ctx.enter_context(tc.tile_pool(name="pO", bufs=3))

    # ---- input loads: sync queue feeds k/v path, gpsimd queue the rest ----
    def load4(pool, dram, cols, nm, eng, col_chunks=None):
        """[512, cols] DRAM -> [128, 4*cols] tile; 4 contraction-chunk views."""
        main = pool.tile([128, 4 * cols], BF16, name=f"{nm}m", tag=f"{nm}m")
        mv = main[:].rearrange("p (ch c) -> p ch c", ch=4)
        for n0, ns in (col_chunks or [(0, cols)]):
            eng.dma_start(
                mv[:, :, n0:n0 + ns],
                dram[0:C, n0:n0 + ns].rearrange("(ch p) c -> p ch c", p=128))
        return [main[:, ci * cols:(ci + 1) * cols] for ci in range(4)]

    wks = load4(wpool, wk_d, 256, "wk", nc.sync)
    nkch = _chunks(NK, 512)
    xks_tile = xpool.tile([128, 4 * NK], BF16, name="xkm", tag="xkm")
    xks_v = xks_tile[:].rearrange("p (ch c) -> p ch c", ch=4)
    n0, ns = nkch[0]
    nc.sync.dma_start(xks_v[:, :, n0:n0 + ns],
                      xkT_d[0:C, n0:n0 + ns].rearrange("(ch p) c -> p ch c", p=128))
    wvs = load4(wpool, wv_d, 260, "wv", nc.sync)
    for n0, ns in nkch[1:]:
        nc.sync.dma_start(xks_v[:, :, n0:n0 + ns],
                          xkT_d[0:C, n0:n0 + ns].rearrange("(ch p) c -> p ch c", p=128))
    xks = [xks_tile[:, ci * NK:(ci + 1) * NK] for ci in range(4)]

    # q-path on the gpsimd queue: first x chunk + Wq, then small tensors;
    # the bulky remaining x chunks and Wo are deferred behind the k-path
    xs_tile = xpool.tile([128, 4 * T], BF16, name="xm", tag="xm")
    xs_v = xs_tile[:].rearrange("p (ch c) -> p ch c", ch=4)

    def x_chunk(eng, n0, ns):
        eng.dma_start(xs_v[:, :, n0:n0 + ns],
                      xT_d[0:C, n0:n0 + ns].rearrange("(ch p) c -> p ch c", p=128))

    x_chunk(nc.gpsimd, 0, 512)
    wqs = load4(wpool, wq_d, 256, "wq", nc.gpsimd)
    xs = [xs_tile[:, ci * T:(ci + 1) * T] for ci in range(4)]
    xk_aug = xpool.tile([1, NK], BF16, name="xka", tag="xka")
    nc.gpsimd.dma_start(xk_aug[:], xkT_d[C:C + 1, :])
    wv_aug = wpool.tile([1, 260], BF16, name="wva", tag="wva")
    nc.gpsimd.dma_start(wv_aug[:], wv_d[C:C + 1, :])
    cdrb = wpool.tile([128, max(NBT, 1)], F32, name="cdrb", tag="cdrb")
    nc.gpsimd.dma_start(cdrb[:], cdrb_d[:])
    x_chunk(nc.gpsimd, 512, 512)
    x_chunk(nc.sync, 1024, 512)
    wo_all = wpool.tile([128, 1024], BF16, name="wo", tag="wo")
    nc.gpsimd.dma_start(wo_all[:].rearrange("p (g c) -> p g c", g=2),
                        wo_d[:].rearrange("(g p) c -> p g c", p=128))
    wo = [wo_all[:, 0:512], wo_all[:, 512:1024]]
    x_chunk(nc.sync, 1536, 512)

    # selector matrix for denominator broadcast: E.T @ cu = row 64 of cu
    # replicated across 64 psum partitions
    esel = wpool.tile([65, 64], BF16, name="esel", tag="esel")
    nc.vector.memset(esel[:], 0.0)
    nc.vector.memset(esel[64:65, :], 1.0)

    # ---- persistent activation tiles ------------------------------------
    qT = [qkv.tile([128, T], BF16, name=f"q{p}", tag=f"q{p}") for p in range(2)]
    kT = [qkv.tile([128, NK], BF16, name=f"k{p}", tag=f"k{p}") for p in range(2)]
    v_sb = qkv.tile([128, NKT * 260], BF16, name="v", tag="v")
    ctxn = [qkv.tile([128, T], BF16, name=f"ctxn{p}", tag=f"ctxn{p}")
            for p in range(2)]

    # ---- projection emitters --------------------------------------------
    def k_proj(pp):
        for n0, ns in _chunks(NK, 512):
            mt = psM.tile([128, 512], F32, name="m", tag="m")
            for ci in range(4):
                nc.tensor.matmul(
                    mt[:, :ns],
                    wks[ci][:, pp * 128:(pp + 1) * 128],
                    xks[ci][:, n0:n0 + ns],
                    start=(ci == 0), stop=(ci == 3))
            nc.vector.tensor_copy(kT[pp][:, n0:n0 + ns], mt[:, :ns])

    def v_proj(kt):
        mt = psM.tile([128, 512], F32, name="m", tag="m")
        for ci in range(4):
            nc.tensor.matmul(
                mt[:, 0:260],
                xks[ci][:, kt * 128:(kt + 1) * 128],
                wvs[ci][:],
                start=(ci == 0), stop=False)
        nc.tensor.matmul(
            mt[:, 0:260],
            xk_aug[0:1, kt * 128:(kt + 1) * 128],
            wv_aug[:],
            start=False, stop=True)
        nc.vector.tensor_copy(v_sb[:, kt * 260:(kt + 1) * 260], mt[:, 0:260])

    def q_proj(qc, pp):
        mt = psM.tile([128, 512], F32, name="m", tag="m")
        for ci in range(4):
            nc.tensor.matmul(
                mt[:],
                wqs[ci][:, pp * 128:(pp + 1) * 128],
                xs[ci][:, qc * 512:(qc + 1) * 512],
                start=(ci == 0), stop=(ci == 3))
        nc.vector.tensor_copy(qT[pp][:, qc * 512:(qc + 1) * 512], mt[:])

    def out_proj(qc, tqs):
        for tq in tqs:
            t0 = qc * 512 + tq * 128
            cp = psM.tile([128, 512], F32, name="m", tag="m")
            nc.tensor.matmul(cp[:], ctxn[0][:, t0:t0 + 128], wo[0],
                             start=True, stop=False)
            nc.tensor.matmul(cp[:], ctxn[1][:, t0:t0 + 128], wo[1],
                             start=False, stop=True)
            ot = pO.tile([128, 512], F32, name="ot", tag="o")
            nc.vector.tensor_copy(ot[:], cp[:])
            nc.sync.dma_start(out_d[t0:t0 + 128, :], ot[:])

    # ---- attention ------------------------------------------------------
    # Split into a scores+exp pass (A) and a ctx+normalize pass (B). A(i)
    # and B(i-1) are emitted interleaved: while phase i's exp activates
    # pace the scalar engine, the previous phase's ctx matmuls (dep-free,
    # P is already in SBUF) keep the PE array dense.
    # P-slot map. pair1 (and pair0's dual region kt<BIAS0): slot 2kt+h so a
    # single [128,1024] act covers both heads of one ktile. pair0's tail:
    # h0-biased singles get slots [2*BIAS0, 2*BIAS0+NBT); h1 tiles kt>=BIAS0
    # are packed CONSECUTIVELY so two ktiles share one act.
    def pslot(pair, kt, h):
        if pair == 1 or kt < BIAS0:
            return 2 * kt + h
        if h == 0:
            return 2 * BIAS0 + (kt - BIAS0)
        return 2 * BIAS0 + NBT + (kt - BIAS0)

    def attn_scores(qc, pair, Pt):
        q0 = qc * 512

        def score_mm(sp, half, pair_, kt, h):
            r0 = h * 64
            nc.tensor.matmul(
                sp[:, half * 512:(half + 1) * 512],
                kT[pair_][r0:r0 + 64, kt * 128:(kt + 1) * 128],
                qT[pair_][r0:r0 + 64, q0:q0 + 512],
                start=True, stop=True,
                tile_position=(r0, 0))

        dual_end = NKT if pair == 1 else BIAS0
        for kt in range(dual_end):
            sp = psS.tile([128, 1024], F32, name="S", tag="S")
            score_mm(sp, 0, pair, kt, 0)
            score_mm(sp, 1, pair, kt, 1)
            nc.scalar.activation(
                Pt[:, pslot(pair, kt, 0) * 512:(pslot(pair, kt, 1) + 1) * 512],
                sp[:], Exp, scale=EXP_SCALE)
            yield
        if pair == 0:
            # h0 biased singles
            for kt in range(BIAS0, NKT_CDR):
                sp = psS.tile([128, 1024], F32, name="S", tag="S")
                score_mm(sp, 0, 0, kt, 0)
                s0 = pslot(0, kt, 0)
                nc.scalar.activation(
                    Pt[:, s0 * 512:(s0 + 1) * 512], sp[:, 0:512],
                    Exp, bias=cdrb[:, kt - BIAS0:kt - BIAS0 + 1],
                    scale=EXP_SCALE)
                yield
            # h1 tail tiles, two ktiles per act
            for kt0 in range(BIAS0, NKT, 2):
                kts = [kt for kt in (kt0, kt0 + 1) if kt < NKT]
                sp = psS.tile([128, 1024], F32, name="S", tag="S")
                for j, kt in enumerate(kts):
                    score_mm(sp, j, 0, kt, 1)
                s0 = pslot(0, kts[0], 1)
                nc.scalar.activation(
                    Pt[:, s0 * 512:(s0 + len(kts)) * 512],
                    sp[:, 0:len(kts) * 512], Exp, scale=EXP_SCALE)
                yield

    def attn_ctx(qc, pair, Pt):
        q0 = qc * 512
        nts = [NKT_CDR if pair == 0 else NKT, NKT]
        cp = [psC.tile([65, 512], F32, name=f"c{h}", tag="ctx")
              for h in range(2)]
        done = [0, 0]
        for kt in range(NKT):
            hs = [h for h in range(2) if kt < nts[h]]
            for h in hs:
                head = 2 * pair + h
                s0 = pslot(pair, kt, h)
                nc.tensor.matmul(
                    cp[h][:],
                    v_sb[:, kt * 260 + head * 65:kt * 260 + (head + 1) * 65],
                    Pt[:, s0 * 512:(s0 + 1) * 512],
                    start=(done[h] == 0), stop=(done[h] + 1 == nts[h]))
                done[h] += 1
                yield
                if done[h] == nts[h]:
                    # normalize: PE-broadcast the denominator row, then DVE
                    # fast-reciprocal + multiply at 64-partition width
                    r0 = h * 64
                    cu = pR.tile([65, 512], BF16, name="cu", tag="cu")
                    nc.vector.tensor_copy(cu[:], cp[h][:])
                    dn = psM.tile([128, 512], F32, name="m", tag="m")
                    nc.tensor.matmul(dn[0:64, :], esel[:], cu[:],
                                     start=True, stop=True)
                    rb = pR.tile([64, 512], F32, name="rb", tag="rb")
                    nc.vector.reciprocal_approx_fast(rb[:], dn[0:64, :])
                    nc.vector.tensor_mul(
                        ctxn[pair][r0:r0 + 64, q0:q0 + 512],
                        cu[0:64, :], rb[:])
                    yield

    # ---- emission order -------------------------------------------------
    # Instruction order per engine is STATIC: a DMA-blocked matmul blocks
    # everything emitted after it on the PE. The emission order below is
    # hand-matched to DMA arrival.
    # PE warmup: one big dummy-matmul block warms the HAM clock gate while
    # the first input DMAs land.
    wps = psM.tile([128, 512], F32, name="m", tag="m")
    for _ in range(130):
        nc.tensor.matmul(wps[0:64, 0:64], esel[:], esel[:],
                         start=True, stop=True)
    nc.vector.tensor_copy(pR.tile([64, 64], F32, name="wd", tag="wd")[:],
                          wps[0:64, 0:64])

    def take(gen, n):
        for _ in range(n):
            if next(gen, StopIteration) is StopIteration:
                return False
        return True

    # pair 1 before pair 0 within each qc: the final phase is then the CDR
    # pair whose h0 stream ends early, shrinking the serial tail
    phases = [(qc, pair) for qc in range(4) for pair in (1, 0)]
    NPH = len(phases)
    Bs = [None] * NPH
    drained = [0] * NPH

    def mk_phase(i):
        qc, pair = phases[i]
        Pt = pP.tile([128, NKT * 1024], BF16, name="P", tag="P")
        Bs[i] = attn_ctx(qc, pair, Pt)
        return attn_scores(qc, pair, Pt)

    def drain_one(j, step_cap=None):
        if Bs[j] is None:
            return False
        if step_cap is not None and drained[j] >= step_cap:
            return False
        if next(Bs[j], StopIteration) is StopIteration:
            Bs[j] = None
            return False
        drained[j] += 1
        return True

    # --- prologue, software-pipelined against the k-path DMA chunks ------
    def k_proj_chunk(pp, n0, ns):
        mt = psM.tile([128, 512], F32, name="m", tag="m")
        for ci in range(4):
            nc.tensor.matmul(
                mt[:, :ns],
                wks[ci][:, pp * 128:(pp + 1) * 128],
                xks[ci][:, n0:n0 + ns],
                start=(ci == 0), stop=(ci == 3))
        nc.vector.tensor_copy(kT[pp][:, n0:n0 + ns], mt[:, :ns])

    A0 = mk_phase(0)
    p0 = phases[0][1]
    first = True
    for n0, ns in _chunks(NK, 512):
        k_proj_chunk(p0, n0, ns)
        k_proj_chunk(1 - p0, n0, ns)
        if first:
            q_proj(0, p0)
            q_proj(0, 1 - p0)
            first = False
        take(A0, ns // 128)          # scores for the ktiles this chunk covers
        for kt in range(n0 // 128, (n0 + ns) // 128):
            v_proj(kt)

    # --- main phase loop -------------------------------------------------
    # Per A-step: drain the previous phase's ctx (fine-grained yields), then
    # head-start this phase's own ctx. The last two A-steps emit nothing
    # else (clean runway so the next phase's first scores issue promptly).
    for i in range(1, NPH):
        A = mk_phase(i)
        ny = (BIAS0 + NBT + (NKT - BIAS0 + 1) // 2
              if phases[i][1] == 0 else NKT)
        runway = 3 if phases[i][1] == 0 else 2
        for step, _ in enumerate(A):
            budget = 0 if step >= ny - runway else 3
            if i >= 2:
                while budget and drain_one(i - 2):
                    budget -= 1
            while budget and drain_one(i - 1):
                budget -= 1
            while budget and drain_one(i, step_cap=2 * step):
                budget -= 1
            if step == 3 and i + 1 < NPH and phases[i + 1][0] >= 1:
                q_proj(*phases[i + 1])
            if i >= 3 and 1 <= step < 3:
                qd = (i - 3) // 2
                tqs = (0, 1) if (i - 3) % 2 == 0 else (2, 3)
                out_proj(qd, (tqs[step - 1],))
            if i == NPH - 1 and step == 5:
                out_proj(2, (2,))
                out_proj(2, (3,))
        if i >= 2:
            while drain_one(i - 2):
                pass
    for j in (NPH - 2, NPH - 1):
        while drain_one(j):
            pass
    out_proj(3, range(4))


# ---------------------------------------------------------------------------
# host side
# ---------------------------------------------------------------------------

def _host_prep(x, mask, cdrs_score, Wq, bq, Wk, bk, Wv, bv, Wo, bo):
    x = np.ascontiguousarray(np.asarray(x, np.float32))
    mask = np.asarray(mask)
    cdrs = np.asarray(cdrs_score)
    Wq = np.asarray(Wq, np.float32)
    Wk = np.asarray(Wk, np.float32)
    Wv = np.asarray(Wv, np.float32)
    Wo = np.asarray(Wo, np.float32)
    bv = np.asarray(bv, np.float32)
    assert np.abs(np.asarray(bq)).max() < 1e-6, "nonzero bq unsupported"
    assert np.abs(np.asarray(bk)).max() < 1e-6, "nonzero bk unsupported"

    gathers = []
    for b in range(B):
        valid = mask[b] == 1
        cdrv = valid & (cdrs[b] == 1) if np.any(cdrs[b] == 1) else valid
        regv = valid & ~cdrv
        gathers.append((np.nonzero(cdrv)[0], np.nonzero(regv)[0]))
    ncdrs = [len(g[0]) for g in gathers]
    valids = [len(g[0]) + len(g[1]) for g in gathers]
    NKT = max(1, math.ceil(max(valids) / 128))
    NK = NKT * 128
    NKT_CDR = max(1, math.ceil(max(ncdrs) / 128))
    BIAS0 = min(ncdrs) // 128
    NBT = NKT_CDR - BIAS0

    # per-group weight bundles (shared across samples)
    wbund = []
    for g in range(2):
        heads = [g, g + 2, g + 4, g + 6]
        dims = np.concatenate([np.arange(h * D, (h + 1) * D) for h in heads])
        wq_c = Wq[:, dims]
        wk_c = Wk[:, dims]
        wv_cols = []
        for h in heads:
            hd = np.arange(h * D, (h + 1) * D)
            wv = np.concatenate([Wv[:, hd], bv[hd][None, :]], axis=0)
            sel = np.zeros((C + 1, 1), np.float32)
            sel[C, 0] = 1.0
            wv_cols.append(np.concatenate([wv, sel], axis=1))
        wv_aug = np.concatenate(wv_cols, axis=1)
        wo_rows = Wo[dims, :]
        wbund.append(tuple(
            np.ascontiguousarray(w.astype(ml_dtypes.bfloat16))
            for w in (wq_c, wk_c, wv_aug, wo_rows)))

    in_maps = []
    for b in range(B):
        idx_cdr, idx_reg = gathers[b]
        nv = len(idx_cdr) + len(idx_reg)
        xk = np.zeros((NK, C), np.float32)
        xk[:len(idx_cdr)] = x[b, idx_cdr]
        xk[len(idx_cdr):nv] = x[b, idx_reg]
        ones_row = np.zeros((1, NK), np.float32)
        ones_row[0, :nv] = 1.0
        xkT_aug = np.ascontiguousarray(
            np.concatenate([xk.T, ones_row], axis=0))
        xT_bf = np.ascontiguousarray(x[b].T.astype(ml_dtypes.bfloat16))
        xkT_bf = np.ascontiguousarray(xkT_aug.astype(ml_dtypes.bfloat16))
        cdrb = np.zeros((128, max(NBT, 1)), np.float32)
        for t in range(NBT):
            keys = (BIAS0 + t) * 128 + np.arange(128)
            cdrb[:, t] = np.where(keys < len(idx_cdr), 0.0, MASK_BIAS)
        for g in range(2):
            wq_c, wk_c, wv_aug, wo_rows = wbund[g]
            in_maps.append({
                "xT": xT_bf, "xkT": xkT_bf,
                "Wq": wq_c, "Wk": wk_c, "Wv": wv_aug, "Wo": wo_rows,
                "cdrb": cdrb,
            })
    return in_maps, NKT, NKT_CDR, BIAS0, NBT


def kernel(**inputs) -> np.ndarray:
    global LAST_RESULTS
    in_maps, NKT, NKT_CDR, BIAS0, NBT = _host_prep(**inputs)

    key = (NKT, NKT_CDR, BIAS0, NBT)
    nc = _PROGRAM_CACHE.get(key)
    if nc is None:
        nc = _build_program(NKT, NKT_CDR, BIAS0, NBT)
        _PROGRAM_CACHE[key] = nc

    res = run_bass_kernel_spmd(nc, in_maps, core_ids=list(range(8)))
    LAST_RESULTS = res

    bo = np.asarray(inputs["bo"], np.float32)
    out = np.empty((B, T, C), np.float32)
    for b in range(B):
        out[b] = res.results[2 * b]["out"] + res.results[2 * b + 1]["out"] + bo[None, :]
    return out


# revision 40
# speedup vs baseline: 1.1803x; 1.0052x over previous
"""Trainium2 Bass kernel for nn_CDRsAttention (sparse multi-head attention
with padding mask + CDR key mask on the first 2 heads).

Sharding: 8 cores = 4 samples (B) x 2 head groups. Core (b, g) computes
heads [g, g+2, g+4, g+6] of sample b (exactly one CDR head each), producing
a partial output ctx_heads @ Wo_rows; the host sums the two partials + bo.

Host-side prep (pure numpy, cheap):
  - per-sample key gather: only keys with mask==1 participate, CDR-valid
    keys first, then regular keys, zero-padded to NK = 128*ceil(max valid).
    No inter-region padding: the CDR head attends tiles [0, NKT_CDR) and
    masks intruding regular keys via a per-partition additive bias (-30)
    fed to the exp activation for the boundary tiles.
  - xkT carries one extra row (valid-key indicator) that flows through
    augmented Wv selector columns so v column h*65+64 is the indicator row,
    making ctx^T row 64 the softmax denominator (padded keys drop out).
  - q/k biases are zero, so their projections contract over exactly 512
    rows (no augmented row).

Device per core (bf16 matmuls):
  qT/kT/v projections -> per head: S^T = kT_tile^T @ qT (keys on psum
  partitions), P = exp(S^T/8) on ScalarE straight out of PSUM (pairs of
  ktiles per activate), ctx^T accumulated as v_aug^T @ P, then a
  fast-reciprocal of the denominator row, a PE broadcast matmul (f32r),
  a DVE normalization multiply, and the output projection
  out = ctx_norm^T.T @ Wo_rows streamed straight to DRAM per 128-query
  slab. q/out projections are interleaved as PE filler between attention
  groups to keep the PE array dense and HAM-warm.
"""
import math
from contextlib import ExitStack

import ml_dtypes
import numpy as np

import concourse.bass as bass
import concourse.mybir as mybir
import concourse.tile as tile
from concourse import bacc
from concourse.bass_utils import run_bass_kernel_spmd

B, T, C, H, D = 4, 2048, 512, 8, 64
F32 = mybir.dt.float32
F32R = mybir.dt.float32r
BF16 = mybir.dt.bfloat16
EXP_SCALE = 1.0 / 8.0  # 1/sqrt(D)
MASK_BIAS = -30.0

_PROGRAM_CACHE: dict = {}
LAST_RESULTS = None  # BassKernelResults of the most recent kernel() call


def _chunks(total, step):
    return [(i, min(step, total - i)) for i in range(0, total, step)]


def _build_program(NKT, NKT_CDR, BIAS0, NBT):
    NK = NKT * 128
    nc = bacc.Bacc("TRN2", target_bir_lowering=False, debug=False, num_devices=8)
    xT_d = nc.dram_tensor("xT", [C, T], BF16, kind="ExternalInput").ap()
    xkT_d = nc.dram_tensor("xkT", [C + 1, NK], BF16, kind="ExternalInput").ap()
    wq_d = nc.dram_tensor("Wq", [C, 256], BF16, kind="ExternalInput").ap()
    wk_d = nc.dram_tensor("Wk", [C, 256], BF16, kind="ExternalInput").ap()
    wv_d = nc.dram_tensor("Wv", [C + 1, 260], BF16, kind="ExternalInput").ap()
    wo_d = nc.dram_tensor("Wo", [256, 512], BF16, kind="ExternalInput").ap()
    cdrb_d = nc.dram_tensor("cdrb", [128, max(NBT, 1)], F32,
                            kind="ExternalInput").ap()
    out_d = nc.dram_tensor("out", [T, 512], F32, kind="ExternalOutput").ap()

    with tile.TileContext(nc) as tc:
        with ExitStack() as ctx:
            _body(ctx, tc, xT_d, xkT_d, wq_d, wk_d, wv_d, wo_d, cdrb_d, out_d,
                  NK, NKT, NKT_CDR, BIAS0, NBT)
    nc.compile()
    return nc


def _body(ctx, tc, xT_d, xkT_d, wq_d, wk_d, wv_d, wo_d, cdrb_d, out_d,
          NK, NKT, NKT_CDR, BIAS0, NBT):
    nc = tc.nc
    Exp = mybir.ActivationFunctionType.Exp

    wpool = ctx.enter_context(tc.tile_pool(name="w", bufs=1))
    xpool = ctx.enter_context(tc.tile_pool(name="x", bufs=1))
    qkv = ctx.enter_context(tc.tile_pool(name="qkv", bufs=1))
    psS = ctx.enter_context(tc.tile_pool(name="psS", bufs=2, space="PSUM"))
    psC = ctx.enter_context(tc.tile_pool(name="psC", bufs=2, space="PSUM"))
    psM = ctx.enter_context(tc.tile_pool(name="psM", bufs=2, space="PSUM"))
    pP = ctx.enter_context(tc.tile_pool(name="pP", bufs=2))
    pR = ctx.enter_context(tc.tile_pool(name="pR", bufs=2))
    pO = ctx.enter_context(tc.tile_pool(name="pO", bufs=3))

    # ---- input loads: sync queue feeds k/v path, gpsimd queue the rest ----
    def load4(pool, dram, cols, nm, eng, col_chunks=None):
        """[512, cols] DRAM -> [128, 4*cols] tile; 4 contraction-chunk views."""
        main = pool.tile([128, 4 * cols], BF16, name=f"{nm}m", tag=f"{nm}m")
        mv = main[:].rearrange("p (ch c) -> p ch c", ch=4)
        for n0, ns in (col_chunks or [(0, cols)]):
            eng.dma_start(
                mv[:, :, n0:n0 + ns],
                dram[0:C, n0:n0 + ns].rearrange("(ch p) c -> p ch c", p=128))
        return [main[:, ci * cols:(ci + 1) * cols] for ci in range(4)]

    wks = load4(wpool, wk_d, 256, "wk", nc.sync)
    nkch = _chunks(NK, 512)
    xks_tile = xpool.tile([128, 4 * NK], BF16, name="xkm", tag="xkm")
    xks_v = xks_tile[:].rearrange("p (ch c) -> p ch c", ch=4)
    n0, ns = nkch[0]
    nc.sync.dma_start(xks_v[:, :, n0:n0 + ns],
                      xkT_d[0:C, n0:n0 + ns].rearrange("(ch p) c -> p ch c", p=128))
    wvs = load4(wpool, wv_d, 260, "wv", nc.sync)
    for n0, ns in nkch[1:]:
        nc.sync.dma_start(xks_v[:, :, n0:n0 + ns],
                          xkT_d[0:C, n0:n0 + ns].rearrange("(ch p) c -> p ch c", p=128))
    xks = [xks_tile[:, ci * NK:(ci + 1) * NK] for ci in range(4)]

    # q-path on the gpsimd queue: first x chunk + Wq, then small tensors;
    # the bulky remaining x chunks and Wo are deferred behind the k-path
    xs_tile = xpool.tile([128, 4 * T], BF16, name="xm", tag="xm")
    xs_v = xs_tile[:].rearrange("p (ch c) -> p ch c", ch=4)

    def x_chunk(eng, n0, ns):
        eng.dma_start(xs_v[:, :, n0:n0 + ns],
                      xT_d[0:C, n0:n0 + ns].rearrange("(ch p) c -> p ch c", p=128))

    x_chunk(nc.gpsimd, 0, 512)
    wqs = load4(wpool, wq_d, 256, "wq", nc.gpsimd)
    xs = [xs_tile[:, ci * T:(ci + 1) * T] for ci in range(4)]
    xk_aug = xpool.tile([1, NK], BF16, name="xka", tag="xka")
    nc.gpsimd.dma_start(xk_aug[:], xkT_d[C:C + 1, :])
    wv_aug = wpool.tile([1, 260], BF16, name="wva", tag="wva")
    nc.gpsimd.dma_start(wv_aug[:], wv_d[C:C + 1, :])
    cdrb = wpool.tile([128, max(NBT, 1)], F32, name="cdrb", tag="cdrb")
    nc.gpsimd.dma_start(cdrb[:], cdrb_d[:])
    x_chunk(nc.gpsimd, 512, 512)
    x_chunk(nc.sync, 1024, 512)
    wo_all = wpool.tile([128, 1024], BF16, name="wo", tag="wo")
    nc.gpsimd.dma_start(wo_all[:].rearrange("p (g c) -> p g c", g=2),
                        wo_d[:].rearrange("(g p) c -> p g c", p=128))
    wo = [wo_all[:, 0:512], wo_all[:, 512:1024]]
    x_chunk(nc.sync, 1536, 512)

    # selector matrix for denominator broadcast: E.T @ cu = row 64 of cu
    # replicated across 64 psum partitions
    esel = wpool.tile([65, 64], BF16, name="esel", tag="esel")
    nc.vector.memset(esel[:], 0.0)
    nc.vector.memset(esel[64:65, :], 1.0)

    # ---- persistent activation tiles ------------------------------------
    qT = [qkv.tile([128, T], BF16, name=f"q{p}", tag=f"q{p}") for p in range(2)]
    kT = [qkv.tile([128, NK], BF16, name=f"k{p}", tag=f"k{p}") for p in range(2)]
    v_sb = qkv.tile([128, NKT * 260], BF16, name="v", tag="v")
    ctxn = [qkv.tile([128, T], BF16, name=f"ctxn{p}", tag=f"ctxn{p}")
            for p in range(2)]

    # ---- projection emitters --------------------------------------------
    def k_proj(pp):
        for n0, ns in _chunks(NK, 512):
            mt = psM.tile([128, 512], F32, name="m", tag="m")
            for ci in range(4):
                nc.tensor.matmul(
                    mt[:, :ns],
                    wks[ci][:, pp * 128:(pp + 1) * 128],
                    xks[ci][:, n0:n0 + ns],
                    start=(ci == 0), stop=(ci == 3))
            nc.vector.tensor_copy(kT[pp][:, n0:n0 + ns], mt[:, :ns])

    def v_proj(kt):
        mt = psM.tile([128, 512], F32, name="m", tag="m")
        for ci in range(4):
            nc.tensor.matmul(
                mt[:, 0:260],
                xks[ci][:, kt * 128:(kt + 1) * 128],
                wvs[ci][:],
                start=(ci == 0), stop=False)
        nc.tensor.matmul(
            mt[:, 0:260],
            xk_aug[0:1, kt * 128:(kt + 1) * 128],
            wv_aug[:],
            start=False, stop=True)
        nc.vector.tensor_copy(v_sb[:, kt * 260:(kt + 1) * 260], mt[:, 0:260])

    def q_proj(qc, pp):
        mt = psM.tile([128, 512], F32, name="m", tag="m")
        for ci in range(4):
            nc.tensor.matmul(
                mt[:],
                wqs[ci][:, pp * 128:(pp + 1) * 128],
                xs[ci][:, qc * 512:(qc + 1) * 512],
                start=(ci == 0), stop=(ci == 3))
        nc.vector.tensor_copy(qT[pp][:, qc * 512:(qc + 1) * 512], mt[:])

    def out_proj(qc, tqs):
        for tq in tqs:
            t0 = qc * 512 + tq * 128
            cp = psM.tile([128, 512], F32, name="m", tag="m")
            nc.tensor.matmul(cp[:], ctxn[0][:, t0:t0 + 128], wo[0],
                             start=True, stop=False)
            nc.tensor.matmul(cp[:], ctxn[1][:, t0:t0 + 128], wo[1],
                             start=False, stop=True)
            ot = pO.tile([128, 512], F32, name="ot", tag="o")
            nc.vector.tensor_copy(ot[:], cp[:])
            nc.sync.dma_start(out_d[t0:t0 + 128, :], ot[:])

    # ---- attention ------------------------------------------------------
    # Split into a scores+exp pass (A) and a ctx+normalize pass (B). A(i)
    # and B(i-1) are emitted interleaved: while phase i's exp activates
    # pace the scalar engine, the previous phase's ctx matmuls (dep-free,
    # P is already in SBUF) keep the PE array dense.
    # P-slot map. pair1 (and pair0's dual region kt<BIAS0): slot 2kt+h so a
    # single [128,1024] act covers both heads of one ktile. pair0's tail:
    # h0-biased singles get slots [2*BIAS0, 2*BIAS0+NBT); h1 tiles kt>=BIAS0
    # are packed CONSECUTIVELY so two ktiles share one act.
    def pslot(pair, kt, h):
        if pair == 1 or kt < BIAS0:
            return 2 * kt + h
        if h == 0:
            return 2 * BIAS0 + (kt - BIAS0)
        return 2 * BIAS0 + NBT + (kt - BIAS0)

    def attn_scores(qc, pair, Pt):
        q0 = qc * 512

        def score_mm(sp, half, pair_, kt, h):
            r0 = h * 64
            nc.tensor.matmul(
                sp[:, half * 512:(half + 1) * 512],
                kT[pair_][r0:r0 + 64, kt * 128:(kt + 1) * 128],
                qT[pair_][r0:r0 + 64, q0:q0 + 512],
                start=True, stop=True,
                tile_position=(r0, 0))

        dual_end = NKT if pair == 1 else BIAS0
        for kt in range(dual_end):
            sp = psS.tile([128, 1024], F32, name="S", tag="S")
            score_mm(sp, 0, pair, kt, 0)
            score_mm(sp, 1, pair, kt, 1)
            nc.scalar.activation(
                Pt[:, pslot(pair, kt, 0) * 512:(pslot(pair, kt, 1) + 1) * 512],
                sp[:], Exp, scale=EXP_SCALE)
            yield
        if pair == 0:
            # h0 biased singles
            for kt in range(BIAS0, NKT_CDR):
                sp = psS.tile([128, 1024], F32, name="S", tag="S")
                score_mm(sp, 0, 0, kt, 0)
                s0 = pslot(0, kt, 0)
                nc.scalar.activation(
                    Pt[:, s0 * 512:(s0 + 1) * 512], sp[:, 0:512],
                    Exp, bias=cdrb[:, kt - BIAS0:kt - BIAS0 + 1],
                    scale=EXP_SCALE)
                yield
            # h1 tail tiles, two ktiles per act
            for kt0 in range(BIAS0, NKT, 2):
                kts = [kt for kt in (kt0, kt0 + 1) if kt < NKT]
                sp = psS.tile([128, 1024], F32, name="S", tag="S")
                for j, kt in enumerate(kts):
                    score_mm(sp, j, 0, kt, 1)
                s0 = pslot(0, kts[0], 1)
                nc.scalar.activation(
                    Pt[:, s0 * 512:(s0 + len(kts)) * 512],
                    sp[:, 0:len(kts) * 512], Exp, scale=EXP_SCALE)
                yield

    def attn_ctx(qc, pair, Pt):
        q0 = qc * 512
        nts = [NKT_CDR if pair == 0 else NKT, NKT]
        cp = [psC.tile([65, 512], F32, name=f"c{h}", tag="ctx")
              for h in range(2)]
        done = [0, 0]
        for kt in range(NKT):
            hs = [h for h in range(2) if kt < nts[h]]
            for h in hs:
                head = 2 * pair + h
                s0 = pslot(pair, kt, h)
                nc.tensor.matmul(
                    cp[h][:],
                    v_sb[:, kt * 260 + head * 65:kt * 260 + (head + 1) * 65],
                    Pt[:, s0 * 512:(s0 + 1) * 512],
                    start=(done[h] == 0), stop=(done[h] + 1 == nts[h]))
                done[h] += 1
                yield
                if done[h] == nts[h]:
                    # normalize: PE-broadcast the denominator row, then DVE
                    # fast-reciprocal + multiply at 64-partition width
                    r0 = h * 64
                    cu = pR.tile([65, 512], BF16, name="cu", tag="cu")
                    nc.vector.tensor_copy(cu[:], cp[h][:])
                    dn = psM.tile([128, 512], F32, name="m", tag="m")
                    nc.tensor.matmul(dn[0:64, :], esel[:], cu[:],
                                     start=True, stop=True)
                    rb = pR.tile([64, 512], F32, name="rb", tag="rb")
                    nc.vector.reciprocal_approx_fast(rb[:], dn[0:64, :])
                    nc.vector.tensor_mul(
                        ctxn[pair][r0:r0 + 64, q0:q0 + 512],
                        cu[0:64, :], rb[:])
                    yield

    # ---- emission order -------------------------------------------------
    # Instruction order per engine is STATIC: a DMA-blocked matmul blocks
    # everything emitted after it on the PE. The emission order below is
    # hand-matched to DMA arrival.
    # PE warmup: one big dummy-matmul block warms the HAM clock gate while
    # the first input DMAs land.
    wps = psM.tile([128, 512], F32, name="m", tag="m")
    for _ in range(150):
        nc.tensor.matmul(wps[0:64, 0:64], esel[:], esel[:],
                         start=True, stop=True)
    nc.vector.tensor_copy(pR.tile([64, 64], F32, name="wd", tag="wd")[:],
                          wps[0:64, 0:64])

    def take(gen, n):
        for _ in range(n):
            if next(gen, StopIteration) is StopIteration:
                return False
        return True

    # pair 1 before pair 0 within each qc: the final phase is then the CDR
    # pair whose h0 stream ends early, shrinking the serial tail
    phases = [(qc, pair) for qc in range(4) for pair in (1, 0)]
    NPH = len(phases)
    Bs = [None] * NPH
    drained = [0] * NPH

    def mk_phase(i):
        qc, pair = phases[i]
        Pt = pP.tile([128, NKT * 1024], BF16, name="P", tag="P")
        Bs[i] = attn_ctx(qc, pair, Pt)
        return attn_scores(qc, pair, Pt)

    def drain_one(j, step_cap=None):
        if Bs[j] is None:
            return False
        if step_cap is not None and drained[j] >= step_cap:
            return False
        if next(Bs[j], StopIteration) is StopIteration:
            Bs[j] = None
            return False
        drained[j] += 1
        return True

    # --- prologue, software-pipelined against the k-path DMA chunks ------
    def k_proj_chunk(pp, n0, ns):
        mt = psM.tile([128, 512], F32, name="m", tag="m")
        for ci in range(4):
            nc.tensor.matmul(
                mt[:, :ns],
                wks[ci][:, pp * 128:(pp + 1) * 128],
                xks[ci][:, n0:n0 + ns],
                start=(ci == 0), stop=(ci == 3))
        nc.vector.tensor_copy(kT[pp][:, n0:n0 + ns], mt[:, :ns])

    A0 = mk_phase(0)
    p0 = phases[0][1]
    first = True
    for n0, ns in _chunks(NK, 512):
        k_proj_chunk(p0, n0, ns)
        k_proj_chunk(1 - p0, n0, ns)
        if first:
            q_proj(0, p0)
            q_proj(0, 1 - p0)
            first = False
        take(A0, ns // 128)          # scores for the ktiles this chunk covers
        for kt in range(n0 // 128, (n0 + ns) // 128):
            v_proj(kt)

    # --- main phase loop -------------------------------------------------
    # Per A-step: drain the previous phase's ctx (fine-grained yields), then
    # head-start this phase's own ctx. The last two A-steps emit nothing
    # else (clean runway so the next phase's first scores issue promptly).
    for i in range(1, NPH):
        A = mk_phase(i)
        ny = (BIAS0 + NBT + (NKT - BIAS0 + 1) // 2
              if phases[i][1] == 0 else NKT)
        for step, _ in enumerate(A):
            budget = 0 if step >= ny - 2 else 3
            if i >= 2:
                while budget and drain_one(i - 2):
                    budget -= 1
            while budget and drain_one(i - 1):
                budget -= 1
            while budget and drain_one(i, step_cap=2 * step):
                budget -= 1
            if step == 3 and i + 1 < NPH and phases[i + 1][0] >= 1:
                q_proj(*phases[i + 1])
            if i >= 3 and 1 <= step < 3:
                qd = (i - 3) // 2
                tqs = (0, 1) if (i - 3) % 2 == 0 else (2, 3)
                out_proj(qd, (tqs[step - 1],))
            if i == NPH - 1 and step == 5:
                out_proj(2, (2,))
                out_proj(2, (3,))
        if i >= 2:
            while drain_one(i - 2):
                pass
    for j in (NPH - 2, NPH - 1):
        while drain_one(j):
            pass
    out_proj(3, range(4))


# ---------------------------------------------------------------------------
# host side
# ---------------------------------------------------------------------------

def _host_prep(x, mask, cdrs_score, Wq, bq, Wk, bk, Wv, bv, Wo, bo):
    x = np.ascontiguousarray(np.asarray(x, np.float32))
    mask = np.asarray(mask)
    cdrs = np.asarray(cdrs_score)
    Wq = np.asarray(Wq, np.float32)
    Wk = np.asarray(Wk, np.float32)
    Wv = np.asarray(Wv, np.float32)
    Wo = np.asarray(Wo, np.float32)
    bv = np.asarray(bv, np.float32)
    assert np.abs(np.asarray(bq)).max() < 1e-6, "nonzero bq unsupported"
    assert np.abs(np.asarray(bk)).max() < 1e-6, "nonzero bk unsupported"

    gathers = []
    for b in range(B):
        valid = mask[b] == 1
        cdrv = valid & (cdrs[b] == 1) if np.any(cdrs[b] == 1) else valid
        regv = valid & ~cdrv
        gathers.append((np.nonzero(cdrv)[0], np.nonzero(regv)[0]))
    ncdrs = [len(g[0]) for g in gathers]
    valids = [len(g[0]) + len(g[1]) for g in gathers]
    NKT = max(1, math.ceil(max(valids) / 128))
    NK = NKT * 128
    NKT_CDR = max(1, math.ceil(max(ncdrs) / 128))
    BIAS0 = min(ncdrs) // 128
    NBT = NKT_CDR - BIAS0

    # per-group weight bundles (shared across samples)
    wbund = []
    for g in range(2):
        heads = [g, g + 2, g + 4, g + 6]
        dims = np.concatenate([np.arange(h * D, (h + 1) * D) for h in heads])
        wq_c = Wq[:, dims]
        wk_c = Wk[:, dims]
        wv_cols = []
        for h in heads:
            hd = np.arange(h * D, (h + 1) * D)
            wv = np.concatenate([Wv[:, hd], bv[hd][None, :]], axis=0)
            sel = np.zeros((C + 1, 1), np.float32)
            sel[C, 0] = 1.0
            wv_cols.append(np.concatenate([wv, sel], axis=1))
        wv_aug = np.concatenate(wv_cols, axis=1)
        wo_rows = Wo[dims, :]
        wbund.append(tuple(
            np.ascontiguousarray(w.astype(ml_dtypes.bfloat16))
            for w in (wq_c, wk_c, wv_aug, wo_rows)))

    in_maps = []
    for b in range(B):
        idx_cdr, idx_reg = gathers[b]
        nv = len(idx_cdr) + len(idx_reg)
        xk = np.zeros((NK, C), np.float32)
        xk[:len(idx_cdr)] = x[b, idx_cdr]
        xk[len(idx_cdr):nv] = x[b, idx_reg]
        ones_row = np.zeros((1, NK), np.float32)
        ones_row[0, :nv] = 1.0
        xkT_aug = np.ascontiguousarray(
            np.concatenate([xk.T, ones_row], axis=0))
        xT_bf = np.ascontiguousarray(x[b].T.astype(ml_dtypes.bfloat16))
        xkT_bf = np.ascontiguousarray(xkT_aug.astype(ml_dtypes.bfloat16))
        cdrb = np.zeros((128, max(NBT, 1)), np.float32)
        for t in range(NBT):
            keys = (BIAS0 + t) * 128 + np.arange(128)
            cdrb[:, t] = np.where(keys < len(idx_cdr), 0.0, MASK_BIAS)
        for g in range(2):
            wq_c, wk_c, wv_aug, wo_rows = wbund[g]
            in_maps.append({
                "xT": xT_bf, "xkT": xkT_bf,
                "Wq": wq_c, "Wk": wk_c, "Wv": wv_aug, "Wo": wo_rows,
                "cdrb": cdrb,
            })
    return in_maps, NKT, NKT_CDR, BIAS0, NBT


def kernel(**inputs) -> np.ndarray:
    global LAST_RESULTS
    in_maps, NKT, NKT_CDR, BIAS0, NBT = _host_prep(**inputs)

    key = (NKT, NKT_CDR, BIAS0, NBT)
    nc = _PROGRAM_CACHE.get(key)
    if nc is None:
        nc = _build_program(NKT, NKT_CDR, BIAS0, NBT)
        _PROGRAM_CACHE[key] = nc

    res = run_bass_kernel_spmd(nc, in_maps, core_ids=list(range(8)))
    LAST_RESULTS = res

    bo = np.asarray(inputs["bo"], np.float32)
    out = np.empty((B, T, C), np.float32)
    for b in range(B):
        out[b] = res.results[2 * b]["out"] + res.results[2 * b + 1]["out"] + bo[None, :]
    return out
